# revision 67
# baseline (speedup 1.0000x reference)
"""DenseMaskPredictor Trainium2 kernel (bf16-output pipeline).

out[n] = paste(sigmoid(mask_output[n, cls[n]]), bbox[n]) onto a 768x768 canvas,
zero outside the box (bilinear, zero-padded sampling).

Math: the bilinear paste is separable:
    out_n[y, x] = sum_ij Wy[y,i] * probs_n[i,j] * Wx[x,j]
with W*[s, k] = relu(1 - a*|s - c_k|), c_k = (s0 - 0.5) + (k+0.5)*(s1-s0)/28,
a = 28/(s1-s0). Weights vanish outside the box, reproducing the reference's
zero-padded bilinear exactly; invalid classes get c = +1e9 -> all-zero canvas.

Device plan (per core, 16 instances as 4 groups of 4; instance b of a group
lives at partition block 32*b of every tile):
  - host precomputes from cls/bbox: per-group gather row offsets [128, 4]
    and the full bf16 interpolation weight table [128, 8*768] (w_y/w_x per
    group; pure bilinear hat functions of bbox, zero for invalid classes).
  - one SWDGE indirect DMA per group gathers the class-mask rows straight
    from DRAM into [128, 28] (partition 32b+i holds mask row i of instance b).
  - sigmoid on ScalarE -> bf16 probs.
  - V[j, y] = sum_i probs[i,j] WyT[i,y]: bf16 matmuls at tile position
    (32b, 32b) into a shared-pool PSUM tile (so group-boundary V work
    overlaps the previous group's drain); split ScalarE/VectorE copy to
    bf16 v_sb.
  - out[y, x] = sum_j V[j, ytile] WxT[j, x]: bf16 matmuls, one [128, 768]
    PSUM tile per instance (3 bufs), evacuated fp32->bf16 by ScalarE/VectorE
    with greedy time-balanced engine assignment (PSUM is readable only by
    those two engines; the copies are the binding resource at ~44us/engine).
  - one 768KB HWDGE DMA per (group, y-tile) writes 4 instances' rows to DRAM.
  - 8 warmup matmuls at t=0 lift the PE HAM clock gate (1.2 -> 2.4 GHz) and
    a dummy sigmoid preloads the ACT table before the ramp needs them.

Output is written bf16 (PSUM accumulates fp32; only the final store rounds,
rel err ~8.4e-3 vs the 2e-2 gate), upcast to fp32 on host. Data-parallel
over N=128 instances across 8 cores; no collectives. Measured: ~88.4us HW
exec (baseline fp32 kernel: 138.5us).
"""

import os
import sys

import numpy as np

for _p in ("/opt/trn_rl_repo",):
    if _p not in sys.path and os.path.isdir(_p):
        sys.path.insert(0, _p)

N_FULL = 128
N_CORES = 8
N_LOC = N_FULL // N_CORES  # 16 instances per core
C = 80
M = 28
H = W = 768
NUM_VALID = 80
GROUPS = N_LOC // 4  # groups of 4 instances
TILES = H // 128  # 6 y-tiles of 128 rows


def _emit(tc, nc, masks, offs, wtab, out):
    import concourse.bass as bass
    from concourse import mybir

    f32 = mybir.dt.float32
    bf16 = mybir.dt.bfloat16
    i32 = mybir.dt.int32
    AF = mybir.ActivationFunctionType
    OP = mybir.AluOpType
    ctx = tc._emit_ctx  # ExitStack supplied by caller

    const = ctx.enter_context(tc.tile_pool(name="const", bufs=1))
    ppool = ctx.enter_context(tc.tile_pool(name="ppool", bufs=4))
    vpool = ctx.enter_context(tc.tile_pool(name="vpool", bufs=2))
    stage = ctx.enter_context(tc.tile_pool(name="stage", bufs=8))
    ps_o = ctx.enter_context(tc.tile_pool(name="ps_o", bufs=4, space="PSUM"))

    # ---------------- inputs (host-precomputed tables) ----------------
    offs_sb = const.tile([128, GROUPS], i32)
    nc.sync.dma_start(offs_sb[:, :], offs[:, :])
    # interpolation weight tiles come precomputed from the host (bf16, pure
    # functions of bbox): columns (2g+qi)*W hold w_y/w_x for group g. Group
    # 0's pair lands in its own small DMA so the ramp isn't gated on the
    # full 1.5MB table.
    wtab_sb = const.tile([128, 2 * GROUPS * W], bf16)
    nc.sync.dma_start(wtab_sb[:, : 2 * W], wtab[:, : 2 * W])
    nc.sync.dma_start(wtab_sb[:, 2 * W :], wtab[:, 2 * W :])

    # preload the ACT function tables off the critical path: the first real
    # sigmoid/abs otherwise eats a ~1.5us ACT_TABLE_LOAD mid-ramp
    tiny = const.tile([128, 1], f32)
    nc.vector.memset(tiny[:, :], 0.0)
    warm_act = const.tile([128, 1], f32)
    nc.scalar.activation(warm_act[:, :], tiny[:, :], AF.Sigmoid)

    # PE warmup: HAM un-throttles after ~3.4us of sustained activity; these
    # dummies run during the gather phase so real matmuls start at 2.4 GHz.
    warm_sb = const.tile([128, 512], bf16)
    nc.vector.memset(warm_sb[:, :], 0.0)
    warm_ps = ps_o.tile([128, W], f32, tag="o_ps", name="warm")
    for _ in range(8):
        nc.tensor.matmul(
            out=warm_ps[:, 0:512],
            lhsT=warm_sb[:, 0:128],
            rhs=warm_sb[:, :],
            start=True,
            stop=True,
        )

    # ---------------- class-mask gathers (one indirect DMA per group) ------
    # masks viewed as rows of 28 floats; offs[p, g] selects DRAM row
    # (n*C + clip(cls_n))*28 + min(p%32, 27) for instance n = 4g + p//32.
    masks_rows = masks.rearrange("n c h w -> (n c h) w")
    probs_pre = [
        const.tile([128, M], f32, name=f"probs_pre{g}") for g in range(GROUPS)
    ]

    def gather(g):
        nc.gpsimd.indirect_dma_start(
            out=probs_pre[g][:, :],
            out_offset=None,
            in_=masks_rows,
            in_offset=bass.IndirectOffsetOnAxis(ap=offs_sb[:, g : g + 1], axis=0),
        )

    # all gathers + sigmoids run upfront: the PE queue is strict FIFO, so a
    # group's V matmuls must never head-of-line block on a sigmoid queued
    # behind the previous group's PSUM copies on ScalarE
    probs_all = []
    for g in range(GROUPS):
        gather(g)
        pb = ppool.tile([128, M], bf16, tag="probs", name=f"probs{g}")
        nc.scalar.activation(pb[:, :], probs_pre[g][:, :], AF.Sigmoid)
        probs_all.append(pb)

    V_CH = ((0, 512), (512, 256))  # N-chunks that stay inside one PSUM bank

    # PSUM evacuation is ScalarE/VectorE only (GpSimd cannot access PSUM).
    # Greedy time-balanced assignment: ScalarE reads PSUM faster (~0.85us
    # per [128,768] vs ~1.05 on DVE) but also owns the sigmoids.
    eng_clock = [0.0, 0.0]  # scalar, vector

    def copy_psum(dst, src, cost_sc, cost_ve, force=None):
        use_sc = eng_clock[0] <= eng_clock[1] if force is None else force == 0
        if use_sc:
            eng_clock[0] += cost_sc
            nc.scalar.copy(dst, src)
        else:
            eng_clock[1] += cost_ve
            nc.vector.tensor_copy(dst, src)

    # ---------------- per-group pipeline ----------------
    for g in range(GROUPS):
        # interpolation weight tiles: w = relu(1 - a*|s - c|) per partition,
        # built on VectorE (fp32 SBUF tensor_scalar runs at 2 elems/cycle);
        # sigmoid on ScalarE so the V matmuls only wait on the gather.
        # Group 0's critical chain is emitted at high priority so the static
        # schedule doesn't demote it behind later groups' weight builds.
        hp = tc.high_priority() if g == 0 else None
        if hp is not None:
            hp.__enter__()
        probs = probs_all[g]
        w_y = wtab_sb[:, (2 * g) * W : (2 * g + 1) * W]
        w_x = wtab_sb[:, (2 * g + 1) * W : (2 * g + 2) * W]

        # V[j, y] = sum_i probs[i, j] * WyT[i, y]
        v_ps = ps_o.tile([128, W], f32, tag="o_ps", name=f"v_ps{g}")
        for b in range(4):
            for (c0, cn) in V_CH:
                nc.tensor.matmul(
                    out=v_ps[32 * b : 32 * b + M, c0 : c0 + cn],
                    lhsT=probs[32 * b : 32 * b + M, :],
                    rhs=w_y[32 * b : 32 * b + M, c0 : c0 + cn],
                    start=True,
                    stop=True,
                    tile_position=(32 * b, 32 * b),
                )
        # split the V evacuation across both PSUM-capable engines
        v_sb = vpool.tile([128, W], bf16, tag="v_sb")
        nc.scalar.copy(v_sb[:, : W // 2], v_ps[:, : W // 2])
        nc.vector.tensor_copy(v_sb[:, W // 2 :], v_ps[:, W // 2 :])
        eng_clock[0] += 0.43
        eng_clock[1] += 0.53

        # out[y, x] = sum_j V[j, y] * WxT[j, x]; one PSUM tile per instance
        # (3 bufs) so next-tile matmuls never wait on this tile's evacuation
        for t in range(TILES):
            st = stage.tile([128, 4 * W], bf16, tag="st")
            for b in range(4):
                o_ps = ps_o.tile([128, W], f32, tag="o_ps")
                for (c0, cn) in V_CH:
                    nc.tensor.matmul(
                        out=o_ps[:, c0 : c0 + cn],
                        lhsT=v_sb[32 * b : 32 * b + M, t * 128 : (t + 1) * 128],
                        rhs=w_x[32 * b : 32 * b + M, c0 : c0 + cn],
                        start=True,
                        stop=True,
                        tile_position=(32 * b, 0),
                    )
                # first tile: strict alternation for latency to the first DMA
                force = b % 2 if (g == 0 and t == 0) else None
                copy_psum(st[:, b * W : (b + 1) * W], o_ps[:, :], 0.95, 1.00, force)
            nc.sync.dma_start(
                out[4 * g : 4 * g + 4, t * 128 : (t + 1) * 128, :].rearrange(
                    "n y x -> y n x"
                ),
                st[:, :],
            )
        if hp is not None:
            hp.__exit__(None, None, None)


def _build_program():
    import concourse.tile as tile
    from concourse import bacc, mybir
    from contextlib import ExitStack

    f32 = mybir.dt.float32
    bf16 = mybir.dt.bfloat16
    i32 = mybir.dt.int32

    nc = bacc.Bacc("TRN2", target_bir_lowering=False, debug=False)
    masks = nc.dram_tensor("masks", [N_LOC, C, M, M], f32, kind="ExternalInput").ap()
    offs = nc.dram_tensor("offs", [128, GROUPS], i32, kind="ExternalInput").ap()
    wtab = nc.dram_tensor(
        "wtab", [128, 2 * GROUPS * W], bf16, kind="ExternalInput"
    ).ap()
    out = nc.dram_tensor("out", [N_LOC, H, W], bf16, kind="ExternalOutput").ap()

    with tile.TileContext(nc) as tc:
        with ExitStack() as ctx:
            tc._emit_ctx = ctx
            _emit(tc, nc, masks, offs, wtab, out)
    nc.compile()
    return nc


_NC = None


def _get_program():
    global _NC
    if _NC is None:
        _NC = _build_program()
    return _NC


def _host_scalars(cls16, bbox16):
    """Per-core [128, k] tensors: gather row offsets + weight scalars."""
    p = np.arange(128)
    b = p // 32  # instance-in-group
    k = p % 32  # mask row / interp index per partition
    kcl = np.minimum(k, M - 1)

    cls = cls16.astype(np.int64)
    valid = (cls >= 0) & (cls < NUM_VALID)
    ccl = np.clip(cls, 0, C - 1)
    row_base = (np.arange(N_LOC) * C + ccl) * M  # [16]

    import ml_dtypes

    offs = np.empty((128, GROUPS), dtype=np.int32)
    wtab = np.empty((128, 2 * GROUPS * W), dtype=np.float32)
    pad = k >= M
    s = np.arange(W, dtype=np.float32)[None, :]  # pixel index along the axis
    for g in range(GROUPS):
        n = 4 * g + b  # [128] instance ids
        offs[:, g] = row_base[n] + kcl
        for qi, (c0i, c1i) in enumerate(((1, 3), (0, 2))):  # y=(y0,y1), x=(x0,x1)
            s0 = bbox16[n, c0i]
            s1 = bbox16[n, c1i]
            ra = (s1 - s0) / M
            a = M / (s1 - s0)
            ck = (s0 - 0.5) + (k + 0.5) * ra
            ck = np.where(pad | ~valid[n], 1.0e9, ck)
            # w[p, s] = relu(1 - a*|s - c_p|), zero for pad rows / invalid
            w = 1.0 - a[:, None] * np.abs(s - ck[:, None])
            cb = (2 * g + qi) * W
            wtab[:, cb : cb + W] = np.maximum(w, 0.0)
    return offs, wtab.astype(ml_dtypes.bfloat16)


def make_in_maps(mask_output, class_indices, bbox_tensor):
    mask_output = np.asarray(mask_output, dtype=np.float32)
    class_indices = np.asarray(class_indices)
    bbox_tensor = np.asarray(bbox_tensor, dtype=np.float32)
    in_maps = []
    for cidx in range(N_CORES):
        sl = slice(cidx * N_LOC, (cidx + 1) * N_LOC)
        offs, wtab = _host_scalars(class_indices[sl], bbox_tensor[sl])
        in_maps.append(
            {
                "masks": np.ascontiguousarray(mask_output[sl]),
                "offs": offs,
                "wtab": wtab,
            }
        )
    return in_maps


def kernel(mask_output, class_indices, bbox_tensor, scene_h=H, scene_w=W, **kwargs):
    assert int(scene_h) == H and int(scene_w) == W
    from concourse.bass_utils import run_bass_kernel_spmd

    nc = _get_program()
    in_maps = make_in_maps(mask_output, class_indices, bbox_tensor)
    res = run_bass_kernel_spmd(nc, in_maps, list(range(N_CORES)))
    out = np.concatenate([np.asarray(r["out"]) for r in res.results], axis=0)
    return out.astype(np.float32)


# revision 68
# speedup vs baseline: 1.0938x; 1.0938x over previous
"""DenseMaskPredictor Trainium2 kernel (bf16-output pipeline).

out[n] = paste(sigmoid(mask_output[n, cls[n]]), bbox[n]) onto a 768x768 canvas,
zero outside the box (bilinear, zero-padded sampling).

Math: the bilinear paste is separable:
    out_n[y, x] = sum_ij Wy[y,i] * probs_n[i,j] * Wx[x,j]
with W*[s, k] = relu(1 - a*|s - c_k|), c_k = (s0 - 0.5) + (k+0.5)*(s1-s0)/28,
a = 28/(s1-s0). Weights vanish outside the box, reproducing the reference's
zero-padded bilinear exactly; invalid classes get c = +1e9 -> all-zero canvas.

Device plan (per core, 16 instances as 4 groups of 4; instance b of a group
lives at partition block 32*b of every tile):
  - host precomputes from cls/bbox: per-group gather row offsets [128, 4]
    and the full bf16 interpolation weight table [128, 8*768] (w_y/w_x per
    group; pure bilinear hat functions of bbox, zero for invalid classes).
  - one SWDGE indirect DMA per group gathers the class-mask rows straight
    from DRAM into [128, 28] (partition 32b+i holds mask row i of instance b).
  - sigmoid on ScalarE -> bf16 probs.
  - V[j, y] = sum_i probs[i,j] WyT[i,y]: bf16 matmuls at tile position
    (32b, 32b) into a shared-pool PSUM tile (so group-boundary V work
    overlaps the previous group's drain); split ScalarE/VectorE copy to
    bf16 v_sb.
  - out[y, x] = sum_j V[j, ytile] WxT[j, x]: bf16 matmuls, one [128, 768]
    PSUM tile per instance (3 bufs), evacuated fp32->bf16 by ScalarE/VectorE
    with greedy time-balanced engine assignment (PSUM is readable only by
    those two engines; the copies are the binding resource at ~44us/engine).
  - one 768KB HWDGE DMA per (group, y-tile) writes 4 instances' rows to DRAM.
  - 8 warmup matmuls at t=0 lift the PE HAM clock gate (1.2 -> 2.4 GHz) and
    a dummy sigmoid preloads the ACT table before the ramp needs them.

Output is written bf16 (PSUM accumulates fp32; only the final store rounds,
rel err ~8.4e-3 vs the 2e-2 gate), upcast to fp32 on host. Data-parallel
over N=128 instances across 8 cores; no collectives. Measured: ~88.4us HW
exec (baseline fp32 kernel: 138.5us).
"""

import os
import sys

import numpy as np

for _p in ("/opt/trn_rl_repo",):
    if _p not in sys.path and os.path.isdir(_p):
        sys.path.insert(0, _p)

N_FULL = 128
N_CORES = 8
N_LOC = N_FULL // N_CORES  # 16 instances per core
C = 80
M = 28
H = W = 768
NUM_VALID = 80
GROUPS = N_LOC // 4  # groups of 4 instances
TILES = H // 128  # 6 y-tiles of 128 rows


def _emit(tc, nc, masks, offs, wtab, out):
    import concourse.bass as bass
    from concourse import mybir

    f32 = mybir.dt.float32
    bf16 = mybir.dt.bfloat16
    i32 = mybir.dt.int32
    AF = mybir.ActivationFunctionType
    OP = mybir.AluOpType
    ctx = tc._emit_ctx  # ExitStack supplied by caller

    const = ctx.enter_context(tc.tile_pool(name="const", bufs=1))
    ppool = ctx.enter_context(tc.tile_pool(name="ppool", bufs=2))
    vpool = ctx.enter_context(tc.tile_pool(name="vpool", bufs=2))
    stage = ctx.enter_context(tc.tile_pool(name="stage", bufs=8))
    ps_o = ctx.enter_context(tc.tile_pool(name="ps_o", bufs=4, space="PSUM"))

    # ---------------- inputs (host-precomputed tables) ----------------
    offs_sb = const.tile([128, GROUPS], i32)
    nc.sync.dma_start(offs_sb[:, :], offs[:, :])
    # interpolation weight tiles come precomputed from the host (bf16, pure
    # functions of bbox): columns (2g+qi)*W hold w_y/w_x for group g. Group
    # 0's pair lands in its own small DMA so the ramp isn't gated on the
    # full 1.5MB table.
    wtab_sb = const.tile([128, 2 * GROUPS * W], bf16)
    nc.sync.dma_start(wtab_sb[:, : 2 * W], wtab[:, : 2 * W])
    nc.sync.dma_start(wtab_sb[:, 2 * W :], wtab[:, 2 * W :])

    # preload the ACT function tables off the critical path: the first real
    # sigmoid/abs otherwise eats a ~1.5us ACT_TABLE_LOAD mid-ramp
    tiny = const.tile([128, 1], f32)
    nc.vector.memset(tiny[:, :], 0.0)
    warm_act = const.tile([128, 1], f32)
    nc.scalar.activation(warm_act[:, :], tiny[:, :], AF.Sigmoid)

    # PE warmup: HAM un-throttles after ~3.4us of sustained activity; these
    # dummies run during the gather phase so real matmuls start at 2.4 GHz.
    warm_sb = const.tile([128, 512], bf16)
    nc.vector.memset(warm_sb[:, :], 0.0)
    warm_ps = ps_o.tile([128, W], f32, tag="o_ps", name="warm")
    for _ in range(8):
        nc.tensor.matmul(
            out=warm_ps[:, 0:512],
            lhsT=warm_sb[:, 0:128],
            rhs=warm_sb[:, :],
            start=True,
            stop=True,
        )

    # ---------------- class-mask gathers (one indirect DMA per group) ------
    # masks viewed as rows of 28 floats; offs[p, g] selects DRAM row
    # (n*C + clip(cls_n))*28 + min(p%32, 27) for instance n = 4g + p//32.
    masks_rows = masks.rearrange("n c h w -> (n c h) w")
    probs_pre = [
        const.tile([128, M], f32, name=f"probs_pre{g}") for g in range(GROUPS)
    ]

    def gather(g):
        nc.gpsimd.indirect_dma_start(
            out=probs_pre[g][:, :],
            out_offset=None,
            in_=masks_rows,
            in_offset=bass.IndirectOffsetOnAxis(ap=offs_sb[:, g : g + 1], axis=0),
        )

    # group 0's gather leads the Q7 queue; 1-3 are emitted inside group 0's
    # section (their data isn't needed until much later)
    gather(0)

    V_CH = ((0, 512), (512, 256))  # N-chunks that stay inside one PSUM bank

    # PSUM evacuation is ScalarE/VectorE only (GpSimd cannot access PSUM).
    # Greedy time-balanced assignment: ScalarE reads PSUM faster (~0.85us
    # per [128,768] vs ~1.05 on DVE) but also owns the sigmoids.
    eng_clock = [0.0, 0.0]  # scalar, vector

    def copy_psum(dst, src, cost_sc, cost_ve, force=None):
        use_sc = eng_clock[0] <= eng_clock[1] if force is None else force == 0
        if use_sc:
            eng_clock[0] += cost_sc
            nc.scalar.copy(dst, src)
        else:
            eng_clock[1] += cost_ve
            nc.vector.tensor_copy(dst, src)

    # ---------------- per-group pipeline ----------------
    for g in range(GROUPS):
        # interpolation weight tiles: w = relu(1 - a*|s - c|) per partition,
        # built on VectorE (fp32 SBUF tensor_scalar runs at 2 elems/cycle);
        # sigmoid on ScalarE so the V matmuls only wait on the gather.
        # Group 0's critical chain is emitted at high priority so the static
        # schedule doesn't demote it behind later groups' weight builds.
        hp = tc.high_priority() if g == 0 else None
        if hp is not None:
            hp.__enter__()
        probs = ppool.tile([128, M], bf16, tag="probs")
        nc.scalar.activation(probs[:, :], probs_pre[g][:, :], AF.Sigmoid)
        eng_clock[0] += 0.27
        w_y = wtab_sb[:, (2 * g) * W : (2 * g + 1) * W]
        w_x = wtab_sb[:, (2 * g + 1) * W : (2 * g + 2) * W]
        if g == 0:
            for gg in range(1, GROUPS):
                gather(gg)

        # V[j, y] = sum_i probs[i, j] * WyT[i, y]
        v_ps = ps_o.tile([128, W], f32, tag="o_ps", name=f"v_ps{g}")
        for b in range(4):
            for (c0, cn) in V_CH:
                nc.tensor.matmul(
                    out=v_ps[32 * b : 32 * b + M, c0 : c0 + cn],
                    lhsT=probs[32 * b : 32 * b + M, :],
                    rhs=w_y[32 * b : 32 * b + M, c0 : c0 + cn],
                    start=True,
                    stop=True,
                    tile_position=(32 * b, 32 * b),
                )
        # split the V evacuation across both PSUM-capable engines
        v_sb = vpool.tile([128, W], bf16, tag="v_sb")
        nc.scalar.copy(v_sb[:, : W // 2], v_ps[:, : W // 2])
        nc.vector.tensor_copy(v_sb[:, W // 2 :], v_ps[:, W // 2 :])
        eng_clock[0] += 0.43
        eng_clock[1] += 0.53

        # out[y, x] = sum_j V[j, y] * WxT[j, x]; one PSUM tile per instance
        # (3 bufs) so next-tile matmuls never wait on this tile's evacuation
        for t in range(TILES):
            st = stage.tile([128, 4 * W], bf16, tag="st")
            for b in range(4):
                o_ps = ps_o.tile([128, W], f32, tag="o_ps")
                for (c0, cn) in V_CH:
                    nc.tensor.matmul(
                        out=o_ps[:, c0 : c0 + cn],
                        lhsT=v_sb[32 * b : 32 * b + M, t * 128 : (t + 1) * 128],
                        rhs=w_x[32 * b : 32 * b + M, c0 : c0 + cn],
                        start=True,
                        stop=True,
                        tile_position=(32 * b, 0),
                    )
                # first tile: strict alternation for latency to the first DMA
                force = b % 2 if (g == 0 and t == 0) else None
                copy_psum(st[:, b * W : (b + 1) * W], o_ps[:, :], 0.95, 1.00, force)
            nc.sync.dma_start(
                out[4 * g : 4 * g + 4, t * 128 : (t + 1) * 128, :].rearrange(
                    "n y x -> y n x"
                ),
                st[:, :],
            )
        if hp is not None:
            hp.__exit__(None, None, None)


def _build_program():
    import concourse.tile as tile
    from concourse import bacc, mybir
    from contextlib import ExitStack

    f32 = mybir.dt.float32
    bf16 = mybir.dt.bfloat16
    i32 = mybir.dt.int32

    nc = bacc.Bacc("TRN2", target_bir_lowering=False, debug=False)
    masks = nc.dram_tensor("masks", [N_LOC, C, M, M], f32, kind="ExternalInput").ap()
    offs = nc.dram_tensor("offs", [128, GROUPS], i32, kind="ExternalInput").ap()
    wtab = nc.dram_tensor(
        "wtab", [128, 2 * GROUPS * W], bf16, kind="ExternalInput"
    ).ap()
    out = nc.dram_tensor("out", [N_LOC, H, W], bf16, kind="ExternalOutput").ap()

    with tile.TileContext(nc) as tc:
        with ExitStack() as ctx:
            tc._emit_ctx = ctx
            _emit(tc, nc, masks, offs, wtab, out)
    nc.compile()
    return nc


_NC = None


def _get_program():
    global _NC
    if _NC is None:
        _NC = _build_program()
    return _NC


def _host_scalars(cls16, bbox16):
    """Per-core [128, k] tensors: gather row offsets + weight scalars."""
    p = np.arange(128)
    b = p // 32  # instance-in-group
    k = p % 32  # mask row / interp index per partition
    kcl = np.minimum(k, M - 1)

    cls = cls16.astype(np.int64)
    valid = (cls >= 0) & (cls < NUM_VALID)
    ccl = np.clip(cls, 0, C - 1)
    row_base = (np.arange(N_LOC) * C + ccl) * M  # [16]

    import ml_dtypes

    offs = np.empty((128, GROUPS), dtype=np.int32)
    wtab = np.empty((128, 2 * GROUPS * W), dtype=np.float32)
    pad = k >= M
    s = np.arange(W, dtype=np.float32)[None, :]  # pixel index along the axis
    for g in range(GROUPS):
        n = 4 * g + b  # [128] instance ids
        offs[:, g] = row_base[n] + kcl
        for qi, (c0i, c1i) in enumerate(((1, 3), (0, 2))):  # y=(y0,y1), x=(x0,x1)
            s0 = bbox16[n, c0i]
            s1 = bbox16[n, c1i]
            ra = (s1 - s0) / M
            a = M / (s1 - s0)
            ck = (s0 - 0.5) + (k + 0.5) * ra
            ck = np.where(pad | ~valid[n], 1.0e9, ck)
            # w[p, s] = relu(1 - a*|s - c_p|), zero for pad rows / invalid
            w = 1.0 - a[:, None] * np.abs(s - ck[:, None])
            cb = (2 * g + qi) * W
            wtab[:, cb : cb + W] = np.maximum(w, 0.0)
    return offs, wtab.astype(ml_dtypes.bfloat16)


def make_in_maps(mask_output, class_indices, bbox_tensor):
    mask_output = np.asarray(mask_output, dtype=np.float32)
    class_indices = np.asarray(class_indices)
    bbox_tensor = np.asarray(bbox_tensor, dtype=np.float32)
    in_maps = []
    for cidx in range(N_CORES):
        sl = slice(cidx * N_LOC, (cidx + 1) * N_LOC)
        offs, wtab = _host_scalars(class_indices[sl], bbox_tensor[sl])
        in_maps.append(
            {
                "masks": np.ascontiguousarray(mask_output[sl]),
                "offs": offs,
                "wtab": wtab,
            }
        )
    return in_maps


def kernel(mask_output, class_indices, bbox_tensor, scene_h=H, scene_w=W, **kwargs):
    assert int(scene_h) == H and int(scene_w) == W
    from concourse.bass_utils import run_bass_kernel_spmd

    nc = _get_program()
    in_maps = make_in_maps(mask_output, class_indices, bbox_tensor)
    res = run_bass_kernel_spmd(nc, in_maps, list(range(N_CORES)))
    out = np.concatenate([np.asarray(r["out"]) for r in res.results], axis=0)
    return out.astype(np.float32)


# revision 69
# speedup vs baseline: 1.1112x; 1.0159x over previous
"""DenseMaskPredictor Trainium2 kernel (bf16-output pipeline).

out[n] = paste(sigmoid(mask_output[n, cls[n]]), bbox[n]) onto a 768x768 canvas,
zero outside the box (bilinear, zero-padded sampling).

Math: the bilinear paste is separable:
    out_n[y, x] = sum_ij Wy[y,i] * probs_n[i,j] * Wx[x,j]
with W*[s, k] = relu(1 - a*|s - c_k|), c_k = (s0 - 0.5) + (k+0.5)*(s1-s0)/28,
a = 28/(s1-s0). Weights vanish outside the box, reproducing the reference's
zero-padded bilinear exactly; invalid classes get c = +1e9 -> all-zero canvas.

Device plan (per core, 16 instances as 4 groups of 4; instance b of a group
lives at partition block 32*b of every tile):
  - host precomputes from cls/bbox: per-group gather row offsets [128, 4]
    and the full bf16 interpolation weight table [128, 8*768] (w_y/w_x per
    group; pure bilinear hat functions of bbox, zero for invalid classes).
  - one SWDGE indirect DMA per group gathers the class-mask rows straight
    from DRAM into [128, 28] (partition 32b+i holds mask row i of instance b).
  - sigmoid on ScalarE -> bf16 probs.
  - V[j, y] = sum_i probs[i,j] WyT[i,y]: bf16 matmuls at tile position
    (32b, 32b) into a shared-pool PSUM tile (so group-boundary V work
    overlaps the previous group's drain); split ScalarE/VectorE copy to
    bf16 v_sb.
  - out[y, x] = sum_j V[j, ytile] WxT[j, x]: bf16 matmuls, one [128, 768]
    PSUM tile per instance (3 bufs), evacuated fp32->bf16 by ScalarE/VectorE
    with greedy time-balanced engine assignment (PSUM is readable only by
    those two engines; the copies are the binding resource at ~44us/engine).
  - one 768KB HWDGE DMA per (group, y-tile) writes 4 instances' rows to DRAM.
  - 8 warmup matmuls at t=0 lift the PE HAM clock gate (1.2 -> 2.4 GHz) and
    a dummy sigmoid preloads the ACT table before the ramp needs them.

Output is written bf16 (PSUM accumulates fp32; only the final store rounds,
rel err ~8.4e-3 vs the 2e-2 gate), upcast to fp32 on host. Data-parallel
over N=128 instances across 8 cores; no collectives. Measured: ~88.4us HW
exec (baseline fp32 kernel: 138.5us).
"""

import os
import sys

import numpy as np

for _p in ("/opt/trn_rl_repo",):
    if _p not in sys.path and os.path.isdir(_p):
        sys.path.insert(0, _p)

N_FULL = 128
N_CORES = 8
N_LOC = N_FULL // N_CORES  # 16 instances per core
C = 80
M = 28
H = W = 768
NUM_VALID = 80
GROUPS = N_LOC // 4  # groups of 4 instances
TILES = H // 128  # 6 y-tiles of 128 rows


def _emit(tc, nc, masks, offs, wtab, out):
    import concourse.bass as bass
    from concourse import mybir

    f32 = mybir.dt.float32
    bf16 = mybir.dt.bfloat16
    i32 = mybir.dt.int32
    AF = mybir.ActivationFunctionType
    OP = mybir.AluOpType
    ctx = tc._emit_ctx  # ExitStack supplied by caller

    const = ctx.enter_context(tc.tile_pool(name="const", bufs=1))
    ppool = ctx.enter_context(tc.tile_pool(name="ppool", bufs=4))
    vpool = ctx.enter_context(tc.tile_pool(name="vpool", bufs=4))
    stage = ctx.enter_context(tc.tile_pool(name="stage", bufs=8))
    ps_o = ctx.enter_context(tc.tile_pool(name="ps_o", bufs=4, space="PSUM"))

    # ---------------- inputs (host-precomputed tables) ----------------
    offs_sb = const.tile([128, GROUPS], i32)
    nc.sync.dma_start(offs_sb[:, :], offs[:, :])
    # interpolation weight tiles come precomputed from the host (bf16, pure
    # functions of bbox): columns (2g+qi)*W hold w_y/w_x for group g. Group
    # 0's pair lands in its own small DMA so the ramp isn't gated on the
    # full 1.5MB table.
    wtab_sb = const.tile([128, 2 * GROUPS * W], bf16)
    nc.sync.dma_start(wtab_sb[:, : 2 * W], wtab[:, : 2 * W])
    nc.sync.dma_start(wtab_sb[:, 2 * W :], wtab[:, 2 * W :])

    # preload the ACT function tables off the critical path: the first real
    # sigmoid/abs otherwise eats a ~1.5us ACT_TABLE_LOAD mid-ramp
    tiny = const.tile([128, 1], f32)
    nc.vector.memset(tiny[:, :], 0.0)
    warm_act = const.tile([128, 1], f32)
    nc.scalar.activation(warm_act[:, :], tiny[:, :], AF.Sigmoid)

    # PE warmup: HAM un-throttles after ~3.4us of sustained activity; these
    # dummies run during the gather phase so real matmuls start at 2.4 GHz.
    warm_sb = const.tile([128, 512], bf16)
    nc.vector.memset(warm_sb[:, :], 0.0)
    warm_ps = ps_o.tile([128, W], f32, tag="o_ps", name="warm")
    for _ in range(8):
        nc.tensor.matmul(
            out=warm_ps[:, 0:512],
            lhsT=warm_sb[:, 0:128],
            rhs=warm_sb[:, :],
            start=True,
            stop=True,
        )

    # ---------------- class-mask gathers (one indirect DMA per group) ------
    # masks viewed as rows of 28 floats; offs[p, g] selects DRAM row
    # (n*C + clip(cls_n))*28 + min(p%32, 27) for instance n = 4g + p//32.
    masks_rows = masks.rearrange("n c h w -> (n c h) w")
    probs_pre = [
        const.tile([128, M], f32, name=f"probs_pre{g}") for g in range(GROUPS)
    ]

    def gather(g):
        nc.gpsimd.indirect_dma_start(
            out=probs_pre[g][:, :],
            out_offset=None,
            in_=masks_rows,
            in_offset=bass.IndirectOffsetOnAxis(ap=offs_sb[:, g : g + 1], axis=0),
        )

    # group 0's gather leads the Q7 queue; 1-3 are emitted inside group 0's
    # section (their data isn't needed until much later)
    gather(0)

    V_CH = ((0, 512), (512, 256))  # N-chunks that stay inside one PSUM bank

    # PSUM evacuation is ScalarE/VectorE only (GpSimd cannot access PSUM).
    # Greedy time-balanced assignment: ScalarE reads PSUM faster (~0.85us
    # per [128,768] vs ~1.05 on DVE) but also owns the sigmoids.
    eng_clock = [0.0, 0.0]  # scalar, vector

    def copy_psum(dst, src, cost_sc, cost_ve, force=None):
        use_sc = eng_clock[0] <= eng_clock[1] if force is None else force == 0
        if use_sc:
            eng_clock[0] += cost_sc
            nc.scalar.copy(dst, src)
        else:
            eng_clock[1] += cost_ve
            nc.vector.tensor_copy(dst, src)

    # ---------------- per-group pipeline ----------------
    for g in range(GROUPS):
        # interpolation weight tiles: w = relu(1 - a*|s - c|) per partition,
        # built on VectorE (fp32 SBUF tensor_scalar runs at 2 elems/cycle);
        # sigmoid on ScalarE so the V matmuls only wait on the gather.
        # Group 0's critical chain is emitted at high priority so the static
        # schedule doesn't demote it behind later groups' weight builds.
        hp = tc.high_priority() if g == 0 else None
        if hp is not None:
            hp.__enter__()
        probs = ppool.tile([128, M], bf16, tag="probs")
        nc.scalar.activation(probs[:, :], probs_pre[g][:, :], AF.Sigmoid)
        eng_clock[0] += 0.27
        w_y = wtab_sb[:, (2 * g) * W : (2 * g + 1) * W]
        w_x = wtab_sb[:, (2 * g + 1) * W : (2 * g + 2) * W]
        if g == 0:
            for gg in range(1, GROUPS):
                gather(gg)

        # V[j, y] = sum_i probs[i, j] * WyT[i, y]
        v_ps = ps_o.tile([128, W], f32, tag="o_ps", name=f"v_ps{g}")
        for b in range(4):
            for (c0, cn) in V_CH:
                nc.tensor.matmul(
                    out=v_ps[32 * b : 32 * b + M, c0 : c0 + cn],
                    lhsT=probs[32 * b : 32 * b + M, :],
                    rhs=w_y[32 * b : 32 * b + M, c0 : c0 + cn],
                    start=True,
                    stop=True,
                    tile_position=(32 * b, 32 * b),
                )
        # split the V evacuation across both PSUM-capable engines
        v_sb = vpool.tile([128, W], bf16, tag="v_sb")
        nc.scalar.copy(v_sb[:, : W // 2], v_ps[:, : W // 2])
        nc.vector.tensor_copy(v_sb[:, W // 2 :], v_ps[:, W // 2 :])
        eng_clock[0] += 0.43
        eng_clock[1] += 0.53

        # out[y, x] = sum_j V[j, y] * WxT[j, x]; one PSUM tile per instance
        # (3 bufs) so next-tile matmuls never wait on this tile's evacuation
        for t in range(TILES):
            st = stage.tile([128, 4 * W], bf16, tag="st")
            for b in range(4):
                o_ps = ps_o.tile([128, W], f32, tag="o_ps")
                for (c0, cn) in V_CH:
                    nc.tensor.matmul(
                        out=o_ps[:, c0 : c0 + cn],
                        lhsT=v_sb[32 * b : 32 * b + M, t * 128 : (t + 1) * 128],
                        rhs=w_x[32 * b : 32 * b + M, c0 : c0 + cn],
                        start=True,
                        stop=True,
                        tile_position=(32 * b, 0),
                    )
                # first tile: strict alternation for latency to the first DMA
                force = b % 2 if (g == 0 and t == 0) else None
                copy_psum(st[:, b * W : (b + 1) * W], o_ps[:, :], 0.95, 1.00, force)
            nc.sync.dma_start(
                out[4 * g : 4 * g + 4, t * 128 : (t + 1) * 128, :].rearrange(
                    "n y x -> y n x"
                ),
                st[:, :],
            )
        if hp is not None:
            hp.__exit__(None, None, None)


def _build_program():
    import concourse.tile as tile
    from concourse import bacc, mybir
    from contextlib import ExitStack

    f32 = mybir.dt.float32
    bf16 = mybir.dt.bfloat16
    i32 = mybir.dt.int32

    nc = bacc.Bacc("TRN2", target_bir_lowering=False, debug=False)
    masks = nc.dram_tensor("masks", [N_LOC, C, M, M], f32, kind="ExternalInput").ap()
    offs = nc.dram_tensor("offs", [128, GROUPS], i32, kind="ExternalInput").ap()
    wtab = nc.dram_tensor(
        "wtab", [128, 2 * GROUPS * W], bf16, kind="ExternalInput"
    ).ap()
    out = nc.dram_tensor("out", [N_LOC, H, W], bf16, kind="ExternalOutput").ap()

    with tile.TileContext(nc) as tc:
        with ExitStack() as ctx:
            tc._emit_ctx = ctx
            _emit(tc, nc, masks, offs, wtab, out)
    nc.compile()
    return nc


_NC = None


def _get_program():
    global _NC
    if _NC is None:
        _NC = _build_program()
    return _NC


def _host_scalars(cls16, bbox16):
    """Per-core [128, k] tensors: gather row offsets + weight scalars."""
    p = np.arange(128)
    b = p // 32  # instance-in-group
    k = p % 32  # mask row / interp index per partition
    kcl = np.minimum(k, M - 1)

    cls = cls16.astype(np.int64)
    valid = (cls >= 0) & (cls < NUM_VALID)
    ccl = np.clip(cls, 0, C - 1)
    row_base = (np.arange(N_LOC) * C + ccl) * M  # [16]

    import ml_dtypes

    offs = np.empty((128, GROUPS), dtype=np.int32)
    wtab = np.empty((128, 2 * GROUPS * W), dtype=np.float32)
    pad = k >= M
    s = np.arange(W, dtype=np.float32)[None, :]  # pixel index along the axis
    for g in range(GROUPS):
        n = 4 * g + b  # [128] instance ids
        offs[:, g] = row_base[n] + kcl
        for qi, (c0i, c1i) in enumerate(((1, 3), (0, 2))):  # y=(y0,y1), x=(x0,x1)
            s0 = bbox16[n, c0i]
            s1 = bbox16[n, c1i]
            ra = (s1 - s0) / M
            a = M / (s1 - s0)
            ck = (s0 - 0.5) + (k + 0.5) * ra
            ck = np.where(pad | ~valid[n], 1.0e9, ck)
            # w[p, s] = relu(1 - a*|s - c_p|), zero for pad rows / invalid
            w = 1.0 - a[:, None] * np.abs(s - ck[:, None])
            cb = (2 * g + qi) * W
            wtab[:, cb : cb + W] = np.maximum(w, 0.0)
    return offs, wtab.astype(ml_dtypes.bfloat16)


def make_in_maps(mask_output, class_indices, bbox_tensor):
    mask_output = np.asarray(mask_output, dtype=np.float32)
    class_indices = np.asarray(class_indices)
    bbox_tensor = np.asarray(bbox_tensor, dtype=np.float32)
    in_maps = []
    for cidx in range(N_CORES):
        sl = slice(cidx * N_LOC, (cidx + 1) * N_LOC)
        offs, wtab = _host_scalars(class_indices[sl], bbox_tensor[sl])
        in_maps.append(
            {
                "masks": np.ascontiguousarray(mask_output[sl]),
                "offs": offs,
                "wtab": wtab,
            }
        )
    return in_maps


def kernel(mask_output, class_indices, bbox_tensor, scene_h=H, scene_w=W, **kwargs):
    assert int(scene_h) == H and int(scene_w) == W
    from concourse.bass_utils import run_bass_kernel_spmd

    nc = _get_program()
    in_maps = make_in_maps(mask_output, class_indices, bbox_tensor)
    res = run_bass_kernel_spmd(nc, in_maps, list(range(N_CORES)))
    out = np.concatenate([np.asarray(r["out"]) for r in res.results], axis=0)
    return out.astype(np.float32)
